# revision 1
# baseline (speedup 1.0000x reference)
"""MoE layer (B=8192, D=2048, H=2048, E=8, top-2) on 8 TRN2 NeuronCores.

Strategy: expert-parallel with host-side routing (the "all-to-all tokens by
routed expert" sharding). kernel() receives the FULL inputs on host, so the
dispatch/combine all-to-all is simply the sharding step:

  1. Gating (0.2% of FLOPs) on host with jax-CPU, bit-matching the
     reference's `x @ gate_W.T + gate_b` -> top_k -> softmax.
  2. For each expert e, gather its routed tokens (~B*K/E = 2048 of them),
     pad to a common capacity C, and hand core e the pair
     (xT_e [D, C], wT_e = expert_W[e].T [D, H]) in bf16.
  3. Each core computes Y_e = X_e @ W_e.T (fp32 accumulate) -- this is
     99.2% of the model FLOPs and 4x less work than the dense einsum.
  4. Host combine: out[b] = sum_k w_k[b] * (Y_{e_k(b)}[col(b)] + b_{e_k(b)}).
"""

import math

import numpy as np

B, D, H, E, TOPK = 8192, 2048, 2048, 8, 2
NCORES = 8

# test.py flips TRACE to profile HW exec time; grading leaves it False.
TRACE = False
last_exec_time_ns = None
last_trace_path = None


def _routing(x, gate_W, gate_b):
    """Reference-exact gating on jax-CPU: logits -> top_k -> softmax."""
    import jax
    import jax.numpy as jnp

    with jax.default_device(jax.devices("cpu")[0]):
        logits = jnp.asarray(x) @ jnp.asarray(gate_W).T + jnp.asarray(gate_b)
        topk_vals, topk_idx = jax.lax.top_k(logits, TOPK)
        topk_w = jax.nn.softmax(topk_vals, axis=1)
    return np.asarray(topk_idx), np.asarray(topk_w, dtype=np.float32)


def _build_bass(C):
    """One Bass program, SPMD across cores: y[C,H] = xT[D,C].T @ wT[D,H]."""
    import concourse.bacc as bacc
    import concourse.mybir as mybir
    import concourse.tile as tile
    from concourse.kernels.tile_matmul import matmul_tile_kernel

    nc = bacc.Bacc("TRN2", target_bir_lowering=False)
    xT = nc.dram_tensor("xT", [D, C], mybir.dt.bfloat16, kind="ExternalInput")
    wT = nc.dram_tensor("wT", [D, H], mybir.dt.bfloat16, kind="ExternalInput")
    y = nc.dram_tensor("y", [C, H], mybir.dt.float32, kind="ExternalOutput")
    with tile.TileContext(nc) as tc:
        matmul_tile_kernel(tc, xT[:], wT[:], y[:])
    nc.compile()
    return nc


def _install_profshim():
    """Register the NTFF profile hook trn_boot couldn't (image's antenv lacks
    axon_hooks) and stub the S3 artifact upload. Only needed when TRACE."""
    import sys
    import types

    import antenv

    if "antenv.axon_hooks" not in sys.modules:
        mod = types.ModuleType("antenv.axon_hooks")
        _hook = [None]
        mod.set_axon_ntff_profile_hook = lambda h: _hook.__setitem__(0, h)
        mod.get_axon_ntff_profile_hook = lambda: _hook[0]
        sys.modules["antenv.axon_hooks"] = mod
        antenv.axon_hooks = mod
        from trn_agent_boot.trn_boot import _ntff_profile_via_ctypes

        mod.set_axon_ntff_profile_hook(
            _ntff_profile_via_ctypes("/opt/axon/libaxon_pjrt.so")
        )
    import concourse.bass_utils as _bu

    _bu.upload_artifacts = lambda tmpdir: f"local:{tmpdir}"


def kernel(x, expert_W, expert_b, gate_W, gate_b):
    global last_exec_time_ns, last_trace_path
    import ml_dtypes

    from concourse.bass_utils import run_bass_kernel_spmd

    x = np.asarray(x, dtype=np.float32)
    expert_W = np.asarray(expert_W, dtype=np.float32)
    expert_b = np.asarray(expert_b, dtype=np.float32)
    gate_W = np.asarray(gate_W, dtype=np.float32)
    gate_b = np.asarray(gate_b, dtype=np.float32)

    topk_idx, topk_w = _routing(x, gate_W, gate_b)

    # Dispatch: token lists per expert (each token appears in exactly TOPK lists).
    tok = [np.nonzero((topk_idx == e).any(axis=1))[0] for e in range(E)]
    counts = np.array([len(t) for t in tok])
    C = max(512, int(math.ceil(counts.max() / 512)) * 512)

    bf16 = ml_dtypes.bfloat16
    xb = x.astype(bf16)  # one RTN cast, reused for all gathers
    in_maps = []
    for e in range(E):
        xTe = np.zeros((D, C), dtype=bf16)
        xTe[:, : counts[e]] = xb[tok[e]].T
        wTe = np.ascontiguousarray(expert_W[e].T.astype(bf16))
        in_maps.append({"xT": xTe, "wT": wTe})

    if TRACE:
        _install_profshim()
    nc = _build_bass(C)
    res = run_bass_kernel_spmd(
        nc, in_maps, list(range(NCORES)), trace=TRACE
    )
    last_exec_time_ns = res.exec_time_ns
    if res.instructions_and_trace:
        last_trace_path = res.instructions_and_trace[1]

    Y = [res.results[e]["y"] for e in range(E)]  # each [C, H] fp32

    # Combine: out[b] = sum_k w_k * (Y_{e_k}[col_k(b)] + b_{e_k})
    colmap = np.zeros((E, B), dtype=np.int64)
    for e in range(E):
        colmap[e, tok[e]] = np.arange(counts[e])
    Ys = np.stack(Y)  # [E, C, H]
    barange = np.arange(B)
    out = np.zeros((B, H), dtype=np.float32)
    for k in range(TOPK):
        ek = topk_idx[:, k]
        ck = colmap[ek, barange]
        out += topk_w[:, k, None] * (Ys[ek, ck, :] + expert_b[ek])
    return out


# revision 2
# speedup vs baseline: 1.0908x; 1.0908x over previous
"""MoE layer (B=8192, D=2048, H=2048, E=8, top-2) on 8 TRN2 NeuronCores.

Strategy: expert-parallel with host-side routing (the "all-to-all tokens by
routed expert" sharding). kernel() receives the FULL inputs on host, so the
dispatch/combine all-to-all is simply the sharding step:

  1. Gating (0.2% of FLOPs) on host with jax-CPU, bit-matching the
     reference's `x @ gate_W.T + gate_b` -> top_k -> softmax.
  2. For each expert e, gather its routed tokens (~B*K/E = 2048 of them),
     pad to a common capacity C, and hand core e the pair
     (xT_e [D, C], wT_e = expert_W[e].T [D, H]) in bf16.
  3. Each core computes Y_e = X_e @ W_e.T (fp32 accumulate) -- this is
     99.2% of the model FLOPs and 4x less work than the dense einsum.
  4. Host combine: out[b] = sum_k w_k[b] * (Y_{e_k(b)}[col(b)] + b_{e_k(b)}).
"""

import math

import numpy as np

B, D, H, E, TOPK = 8192, 2048, 2048, 8, 2
NCORES = 8

# test.py flips TRACE to profile HW exec time; grading leaves it False.
TRACE = False
last_exec_time_ns = None
last_trace_path = None


def _routing(x, gate_W, gate_b):
    """Reference-exact gating on jax-CPU: logits -> top_k -> softmax."""
    import jax
    import jax.numpy as jnp

    with jax.default_device(jax.devices("cpu")[0]):
        logits = jnp.asarray(x) @ jnp.asarray(gate_W).T + jnp.asarray(gate_b)
        topk_vals, topk_idx = jax.lax.top_k(logits, TOPK)
        topk_w = jax.nn.softmax(topk_vals, axis=1)
    return np.asarray(topk_idx), np.asarray(topk_w, dtype=np.float32)


def _build_bass(C):
    """One Bass program, SPMD across cores: y[C,H] = xT[D,C].T @ wT[D,H]."""
    import concourse.bacc as bacc
    import concourse.mybir as mybir
    import concourse.tile as tile
    from concourse.kernels.tile_matmul import matmul_tile_kernel

    nc = bacc.Bacc("TRN2", target_bir_lowering=False)
    xT = nc.dram_tensor("xT", [D, C], mybir.dt.bfloat16, kind="ExternalInput")
    wT = nc.dram_tensor("wT", [D, H], mybir.dt.bfloat16, kind="ExternalInput")
    y = nc.dram_tensor("y", [C, H], mybir.dt.float32, kind="ExternalOutput")
    with tile.TileContext(nc) as tc:
        matmul_tile_kernel(tc, xT[:], wT[:], y[:])
    nc.compile()
    return nc


def _install_profshim():
    """Register the NTFF profile hook trn_boot couldn't (image's antenv lacks
    axon_hooks) and stub the S3 artifact upload. Only needed when TRACE."""
    import sys
    import types

    import antenv

    if "antenv.axon_hooks" not in sys.modules:
        mod = types.ModuleType("antenv.axon_hooks")
        _hook = [None]
        mod.set_axon_ntff_profile_hook = lambda h: _hook.__setitem__(0, h)
        mod.get_axon_ntff_profile_hook = lambda: _hook[0]
        sys.modules["antenv.axon_hooks"] = mod
        antenv.axon_hooks = mod
        from trn_agent_boot.trn_boot import _ntff_profile_via_ctypes

        mod.set_axon_ntff_profile_hook(
            _ntff_profile_via_ctypes("/opt/axon/libaxon_pjrt.so")
        )
    import concourse.bass_utils as _bu

    _bu.upload_artifacts = lambda tmpdir: f"local:{tmpdir}"


def kernel(x, expert_W, expert_b, gate_W, gate_b):
    global last_exec_time_ns, last_trace_path
    import ml_dtypes

    from concourse.bass_utils import run_bass_kernel_spmd

    x = np.asarray(x, dtype=np.float32)
    expert_W = np.asarray(expert_W, dtype=np.float32)
    expert_b = np.asarray(expert_b, dtype=np.float32)
    gate_W = np.asarray(gate_W, dtype=np.float32)
    gate_b = np.asarray(gate_b, dtype=np.float32)

    topk_idx, topk_w = _routing(x, gate_W, gate_b)

    # Dispatch: token lists per expert (each token appears in exactly TOPK lists).
    tok = [np.nonzero((topk_idx == e).any(axis=1))[0] for e in range(E)]
    counts = np.array([len(t) for t in tok])
    # Capacity: smallest padding that still gives matmul_tile_kernel a large
    # M_TILE (384 or 512 divide C) -- M_TILE < 384 would multiply weight
    # re-DMA past the bandwidth budget.
    mx = int(counts.max())
    C = max(384, min(math.ceil(mx / 384) * 384, math.ceil(mx / 512) * 512))

    bf16 = ml_dtypes.bfloat16
    xb = x.astype(bf16)  # one RTN cast, reused for all gathers
    in_maps = []
    for e in range(E):
        xTe = np.zeros((D, C), dtype=bf16)
        xTe[:, : counts[e]] = xb[tok[e]].T
        wTe = np.ascontiguousarray(expert_W[e].T.astype(bf16))
        in_maps.append({"xT": xTe, "wT": wTe})

    if TRACE:
        _install_profshim()
    nc = _build_bass(C)
    res = run_bass_kernel_spmd(
        nc, in_maps, list(range(NCORES)), trace=TRACE
    )
    last_exec_time_ns = res.exec_time_ns
    if res.instructions_and_trace:
        last_trace_path = res.instructions_and_trace[1]

    Y = [res.results[e]["y"] for e in range(E)]  # each [C, H] fp32

    # Combine: out[b] = sum_k w_k * (Y_{e_k}[col_k(b)] + b_{e_k})
    colmap = np.zeros((E, B), dtype=np.int64)
    for e in range(E):
        colmap[e, tok[e]] = np.arange(counts[e])
    Ys = np.stack(Y)  # [E, C, H]
    barange = np.arange(B)
    out = np.zeros((B, H), dtype=np.float32)
    for k in range(TOPK):
        ek = topk_idx[:, k]
        ck = colmap[ek, barange]
        out += topk_w[:, k, None] * (Ys[ek, ck, :] + expert_b[ek])
    return out


# revision 4
# speedup vs baseline: 1.1633x; 1.0665x over previous
"""MoE layer (B=8192, D=2048, H=2048, E=8, top-2) on 8 TRN2 NeuronCores.

Strategy: expert-parallel with host-side routing (the "all-to-all tokens by
routed expert" sharding). kernel() receives the FULL inputs on host, so the
dispatch/combine all-to-all is simply the sharding step:

  1. Gating (0.2% of FLOPs) on host with jax-CPU, bit-matching the
     reference's `x @ gate_W.T + gate_b` -> top_k -> softmax.
  2. For each expert e, gather its routed tokens (~B*K/E = 2048 of them),
     pad to a common capacity C, and hand core e the pair
     (xT_e [D, C], wT_e = expert_W[e].T [D, H]) in bf16.
  3. Each core computes Y_e = X_e @ W_e.T (fp32 accumulate) -- this is
     99.2% of the model FLOPs and 4x less work than the dense einsum.
  4. Host combine: out[b] = sum_k w_k[b] * (Y_{e_k(b)}[col(b)] + b_{e_k(b)}).
"""

import math

import numpy as np

B, D, H, E, TOPK = 8192, 2048, 2048, 8, 2
NCORES = 8

# test.py flips TRACE to profile HW exec time; grading leaves it False.
TRACE = False
last_exec_time_ns = None
last_trace_path = None


def _routing(x, gate_W, gate_b):
    """Reference-exact gating on jax-CPU: logits -> top_k -> softmax."""
    import jax
    import jax.numpy as jnp

    with jax.default_device(jax.devices("cpu")[0]):
        logits = jnp.asarray(x) @ jnp.asarray(gate_W).T + jnp.asarray(gate_b)
        topk_vals, topk_idx = jax.lax.top_k(logits, TOPK)
        topk_w = jax.nn.softmax(topk_vals, axis=1)
    return np.asarray(topk_idx), np.asarray(topk_w, dtype=np.float32)


def _build_bass(seg_rows):
    """One Bass program, SPMD across cores. For segment sizes [s_0..s_{P-1}]
    (summing to C), computes y[off_j:off_j+s_j] = xT[:, off_j:...].T @ w_j
    with a per-core weight tensor per segment."""
    import concourse.bacc as bacc
    import concourse.mybir as mybir
    import concourse.tile as tile
    from concourse.kernels.tile_matmul import matmul_tile_kernel

    C = sum(seg_rows)
    nc = bacc.Bacc("TRN2", target_bir_lowering=False)
    xT = nc.dram_tensor("xT", [D, C], mybir.dt.bfloat16, kind="ExternalInput")
    ws = [
        nc.dram_tensor(f"w{j}", [D, H], mybir.dt.bfloat16, kind="ExternalInput")
        for j in range(len(seg_rows))
    ]
    y = nc.dram_tensor("y", [C, H], mybir.dt.float32, kind="ExternalOutput")
    with tile.TileContext(nc) as tc:
        off = 0
        for j, s in enumerate(seg_rows):
            matmul_tile_kernel(tc, xT[:, off : off + s], ws[j][:], y[off : off + s, :])
            off += s
    nc.compile()
    return nc


def _plan_segments(counts):
    """Choose per-core segment row-sizes (same across cores) and assign every
    expert's token blocks to (core, segment) pieces.

    Returns (seg_rows, pieces) where pieces[e] = ordered [(core, seg, rows)]
    covering counts[e] rows, and no (core, seg) holds more than one expert.
    Falls back to one max-capacity segment per core when the balanced packing
    doesn't fit.
    """
    blocks = [-(-int(n) // 128) for n in counts]
    total = sum(blocks)
    T = -(-total // 8)

    # Candidate per-core block splits: every segment's row count must keep a
    # large M_TILE (divisible by 384 or 512 -> block counts div by 3 or 4).
    def ok(b):
        return b > 0 and (b % 3 == 0 or b % 4 == 0)

    schemes = []
    if ok(T):
        schemes.append([T])
    schemes += [[b1, T - b1] for b1 in range(T - 1, 0, -1) if ok(b1) and ok(T - b1)]

    for seg_blocks in schemes:
        pool = []  # (blocks_capacity, core, seg)
        for c in range(8):
            for j, b in enumerate(seg_blocks):
                pool.append([b, c, j])
        pieces = [[] for _ in range(E)]
        feasible = True
        # Largest experts first; take largest segments first.
        for e in sorted(range(E), key=lambda e: -blocks[e]):
            need = blocks[e]
            while need > 0:
                pool.sort(key=lambda s: -s[0])
                if not pool or pool[0][0] == 0:
                    feasible = False
                    break
                # Prefer an exact fit, else the largest.
                pick = next((s for s in pool if s[0] == need), pool[0])
                take = min(pick[0], need)
                pieces[e].append((pick[1], pick[2], take * 128))
                need -= take
                pool.remove(pick)
            if not feasible:
                break
        if feasible:
            seg_rows = [b * 128 for b in seg_blocks]
            # Trim the last piece of each expert to its true row count.
            for e in range(E):
                used = sum(p[2] for p in pieces[e])
                over = used - int(counts[e])
                if over > 0:
                    c, j, r = pieces[e][-1]
                    pieces[e][-1] = (c, j, r - over)
            return seg_rows, pieces

    # Fallback: single segment of max capacity (always feasible).
    mx = max(512, int(counts.max()))
    C = min(math.ceil(mx / 384) * 384, math.ceil(mx / 512) * 512)
    return [C], [[(e, 0, int(counts[e]))] for e in range(E)]


def _install_profshim():
    """Register the NTFF profile hook trn_boot couldn't (image's antenv lacks
    axon_hooks) and stub the S3 artifact upload. Only needed when TRACE."""
    import sys
    import types

    import antenv

    if "antenv.axon_hooks" not in sys.modules:
        mod = types.ModuleType("antenv.axon_hooks")
        _hook = [None]
        mod.set_axon_ntff_profile_hook = lambda h: _hook.__setitem__(0, h)
        mod.get_axon_ntff_profile_hook = lambda: _hook[0]
        sys.modules["antenv.axon_hooks"] = mod
        antenv.axon_hooks = mod
        from trn_agent_boot.trn_boot import _ntff_profile_via_ctypes

        mod.set_axon_ntff_profile_hook(
            _ntff_profile_via_ctypes("/opt/axon/libaxon_pjrt.so")
        )
    import concourse.bass_utils as _bu

    _bu.upload_artifacts = lambda tmpdir: f"local:{tmpdir}"


def kernel(x, expert_W, expert_b, gate_W, gate_b):
    global last_exec_time_ns, last_trace_path
    import ml_dtypes

    from concourse.bass_utils import run_bass_kernel_spmd

    x = np.asarray(x, dtype=np.float32)
    expert_W = np.asarray(expert_W, dtype=np.float32)
    expert_b = np.asarray(expert_b, dtype=np.float32)
    gate_W = np.asarray(gate_W, dtype=np.float32)
    gate_b = np.asarray(gate_b, dtype=np.float32)

    topk_idx, topk_w = _routing(x, gate_W, gate_b)

    # Dispatch: token lists per expert (each token appears in exactly TOPK lists).
    tok = [np.nonzero((topk_idx == e).any(axis=1))[0] for e in range(E)]
    counts = np.array([len(t) for t in tok])
    seg_rows, pieces = _plan_segments(counts)
    seg_off = np.concatenate([[0], np.cumsum(seg_rows)])
    C = int(seg_off[-1])

    bf16 = ml_dtypes.bfloat16
    xb = x.astype(bf16)  # one RTN cast, reused for all gathers
    wb = [np.ascontiguousarray(expert_W[e].T.astype(bf16)) for e in range(E)]

    # Dispatch per the plan: fill each core's xT columns and pick per-segment
    # weights; record each token's (core, row) for the combine.
    xTs = [np.zeros((D, C), dtype=bf16) for _ in range(NCORES)]
    seg_w = [[0] * len(seg_rows) for _ in range(NCORES)]  # expert id per slot
    core_of = np.zeros((E, B), dtype=np.int64)
    pos_of = np.zeros((E, B), dtype=np.int64)
    for e in range(E):
        cum = 0
        for c, j, rows in pieces[e]:
            t = tok[e][cum : cum + rows]
            lo = int(seg_off[j])
            xTs[c][:, lo : lo + len(t)] = xb[t].T
            seg_w[c][j] = e
            core_of[e, t] = c
            pos_of[e, t] = lo + np.arange(len(t))
            cum += rows

    in_maps = []
    for c in range(NCORES):
        m = {"xT": xTs[c]}
        for j in range(len(seg_rows)):
            m[f"w{j}"] = wb[seg_w[c][j]]
        in_maps.append(m)

    if TRACE:
        _install_profshim()
    nc = _build_bass(seg_rows)
    res = run_bass_kernel_spmd(nc, in_maps, list(range(NCORES)), trace=TRACE)
    last_exec_time_ns = res.exec_time_ns
    if res.instructions_and_trace:
        last_trace_path = res.instructions_and_trace[1]

    Ys = np.stack([res.results[c]["y"] for c in range(NCORES)])  # [8, C, H]

    # Combine: out[b] = sum_k w_k * (Y at (core,row of (e_k, b)) + b_{e_k})
    barange = np.arange(B)
    out = np.zeros((B, H), dtype=np.float32)
    for k in range(TOPK):
        ek = topk_idx[:, k]
        out += topk_w[:, k, None] * (
            Ys[core_of[ek, barange], pos_of[ek, barange], :] + expert_b[ek]
        )
    return out
